# revision 13
# baseline (speedup 1.0000x reference)
import sys

if "/opt/trn_rl_repo" not in sys.path:
    sys.path.insert(0, "/opt/trn_rl_repo")

import numpy as np
from contextlib import ExitStack

import concourse.bass as bass
from concourse import bacc
import concourse.tile as tile
from concourse import mybir
from concourse.bass_utils import run_bass_kernel_spmd

B, C, N, L = 16, 768, 1024, 16
SPC = 2          # samples per core
NCORES = 8
CB = 6           # 128-row chunks of C
JB = 8           # 128-row chunks of N (contraction j)
NBL = 2          # 512-col blocks of N
F32 = mybir.dt.float32
F32R = mybir.dt.float32r
AF = mybir.ActivationFunctionType
ALU = mybir.AluOpType

USE_F32R = True


def _mm(ap):
    return ap.bitcast(F32R) if USE_F32R else ap


def _sl(nb):
    return slice(nb * 512, (nb + 1) * 512)


def build_nc():
    nc = bacc.Bacc(trn_type="TRN2")
    xo_d = nc.declare_dram_parameter("xo", [SPC, C, N], F32, isOutput=False)
    xb_d = nc.declare_dram_parameter("xb", [SPC, C, N], F32, isOutput=False)
    pwt_d = nc.declare_dram_parameter("pwt", [128, CB * L], F32, isOutput=False)
    pbc_d = nc.declare_dram_parameter("pbc", [L, 1], F32, isOutput=False)
    w1t_d = nc.declare_dram_parameter("w1t", [L, C], F32, isOutput=False)
    b1c_d = nc.declare_dram_parameter("b1c", [128, CB], F32, isOutput=False)
    w2c_d = nc.declare_dram_parameter("w2c", [128, CB], F32, isOutput=False)
    b2c_d = nc.declare_dram_parameter("b2c", [1, 1], F32, isOutput=False)
    eye_d = nc.declare_dram_parameter("eye", [128, 128], F32, isOutput=False)
    ones_d = nc.declare_dram_parameter("ones", [128, 1], F32, isOutput=False)
    out_d = nc.declare_dram_parameter("out", [SPC, C, N], F32, isOutput=True)

    with tile.TileContext(nc) as tc, ExitStack() as ctx:
        con = ctx.enter_context(tc.tile_pool(name="con", bufs=1))
        wrk = ctx.enter_context(tc.tile_pool(name="wrk", bufs=2))
        psp = ctx.enter_context(tc.tile_pool(name="psp", bufs=2, space="PSUM"))

        pwt = con.tile([128, CB * L], F32)
        pbc = con.tile([L, 1], F32)
        w1t = con.tile([L, C], F32)
        b1c = con.tile([128, CB], F32)
        w2c = con.tile([128, CB], F32)
        b2c = con.tile([1, 1], F32)
        eye = con.tile([128, 128], F32)
        ones = con.tile([128, 1], F32)
        for t, d in ((pwt, pwt_d), (pbc, pbc_d), (w1t, w1t_d), (b1c, b1c_d),
                     (w2c, w2c_d), (b2c, b2c_d), (eye, eye_d), (ones, ones_d)):
            nc.sync.dma_start(t[:], d[:])
        # f32r-rounded copies (BIR verifier: f32r matmul operands must be
        # produced by a compute op with f32r output dtype)
        pwt_r = con.tile([128, CB * L], F32R)
        w1t_r = con.tile([L, C], F32R)
        w2c_r = con.tile([128, CB], F32R)
        ones_r = con.tile([128, 1], F32R)
        for t, src in ((pwt_r, pwt), (w1t_r, w1t), (w2c_r, w2c), (ones_r, ones)):
            nc.gpsimd.tensor_copy(t[:], src[:])

        for s in range(SPC):
            # ---- zT = proj_w @ xo + b : (16, N); xo streamed + rounded to f32r
            z_ps = [psp.tile([L, 512], F32, name="z_ps", tag="vec") for _ in range(NBL)]
            for cb in range(CB):
                xot = wrk.tile([128, N], F32, name="xo_t", bufs=3)
                nc.sync.dma_start(xot[:], xo_d[s, cb * 128:(cb + 1) * 128, :])
                xor_ = wrk.tile([128, N], F32R, name="xo_r", bufs=3)
                nc.gpsimd.tensor_copy(xor_[:], xot[:])
                for nb in range(NBL):
                    nc.tensor.matmul(
                        z_ps[nb][:],
                        pwt_r[:, cb * L:(cb + 1) * L],
                        xor_[:, _sl(nb)],
                        start=(cb == 0), stop=(cb == CB - 1))
            xb_t = []
            for cb in range(CB):
                xbt = wrk.tile([128, N], F32, name="xb_t", bufs=6)
                nc.sync.dma_start(xbt[:], xb_d[s, cb * 128:(cb + 1) * 128, :])
                xb_t.append(xbt)
            zT = wrk.tile([L, N], F32R, name="zT", bufs=2)
            for nb in range(NBL):
                nc.scalar.activation(zT[:, _sl(nb)], z_ps[nb][:], AF.Identity,
                                     bias=pbc[:], scale=1.0)

            # ---- sq(n) = sum_l zT^2 ; nsq_row = -sq ; r = exp(-sq)
            zsq = wrk.tile([L, N], F32R, name="zsq", bufs=1)
            nc.vector.tensor_mul(zsq[:], zT[:].bitcast(F32), zT[:].bitcast(F32))
            sq_ps = [psp.tile([1, 512], F32, name="sq_ps", tag="vec") for _ in range(NBL)]
            for nb in range(NBL):
                nc.tensor.matmul(sq_ps[nb][:], ones_r[0:L, :], zsq[:, _sl(nb)],
                                 start=True, stop=True)
            nsq_row = wrk.tile([1, N], F32, name="nsq_row", bufs=1)
            for nb in range(NBL):
                nc.vector.tensor_scalar_mul(nsq_row[:, _sl(nb)], sq_ps[nb][:], -1.0)
            nsq_ps = psp.tile([128, JB], F32, name="nsq_ps", tag="tp", bufs=2)
            for j in range(JB):
                nc.tensor.transpose(nsq_ps[:, j:j + 1], nsq_row[:, j * 128:(j + 1) * 128],
                                    eye[0:1, 0:1])
            nsq_col = wrk.tile([128, JB], F32, name="nsq_col", bufs=1)
            nc.scalar.copy(nsq_col[:], nsq_ps[:])
            r_row = wrk.tile([1, N], F32, name="r_row", bufs=1)
            nc.scalar.activation(r_row[:], nsq_row[:], AF.Exp)

            # ---- Gram + Ku[j] = exp(2*G - sq_j)  (row-stabilized kernel)
            ku = [wrk.tile([128, N], F32R, name="ku", bufs=9) for _ in range(JB)]
            for j in range(JB):
                for nb in range(NBL):
                    g_ps = psp.tile([128, 512], F32, name="g_ps", tag="g")
                    nc.tensor.matmul(g_ps[:], zT[:, j * 128:(j + 1) * 128],
                                     zT[:, _sl(nb)], start=True, stop=True)
                    nc.scalar.activation(ku[j][:, _sl(nb)], g_ps[:], AF.Exp,
                                         bias=nsq_col[:, j:j + 1], scale=2.0)

            # ---- qE[n] = sum_j Ku[j, n] (column sums)
            q_ps = [psp.tile([1, 512], F32, name="q_ps", tag="vec") for _ in range(NBL)]
            for nb in range(NBL):
                for j in range(JB):
                    nc.tensor.matmul(q_ps[nb][:], ones_r[:], ku[j][:, _sl(nb)],
                                     start=(j == 0), stop=(j == JB - 1))

            # ---- pi MLP: relu(z @ w1.T + b1) @ w2.T + b2 -> sigmoid
            pi_ps = [psp.tile([1, 512], F32, name="pi_ps", tag="vec") for _ in range(NBL)]
            for cb in range(CB):
                h_ps = [psp.tile([128, 512], F32, name="h_ps", tag="g") for _ in range(NBL)]
                for nb in range(NBL):
                    nc.tensor.matmul(h_ps[nb][:], w1t_r[:, cb * 128:(cb + 1) * 128],
                                     zT[:, _sl(nb)], start=True, stop=True)
                h_sb = wrk.tile([128, N], F32R, name="h_sb", bufs=2)
                for nb in range(NBL):
                    nc.vector.tensor_scalar(h_sb[:, _sl(nb)], h_ps[nb][:],
                                            b1c[:, cb:cb + 1], 0.0,
                                            op0=ALU.add, op1=ALU.max)
                for nb in range(NBL):
                    nc.tensor.matmul(pi_ps[nb][:], w2c_r[:, cb:cb + 1],
                                     h_sb[:, _sl(nb)],
                                     start=(cb == 0), stop=(cb == CB - 1))
            pi_row = wrk.tile([1, N], F32, name="pi_row", bufs=1)
            for nb in range(NBL):
                nc.scalar.activation(pi_row[:, _sl(nb)], pi_ps[nb][:], AF.Sigmoid,
                                     bias=b2c[:], scale=1.0)

            # ---- s = pi / (r * qE)
            q_row = wrk.tile([1, N], F32, name="rtmp", bufs=2)
            for nb in range(NBL):
                nc.vector.tensor_tensor(q_row[:, _sl(nb)], r_row[:, _sl(nb)],
                                        q_ps[nb][:], op=ALU.mult)
            qr_row = wrk.tile([1, N], F32, name="rtmp", bufs=2)
            nc.vector.reciprocal(qr_row[:], q_row[:])
            s_row = wrk.tile([1, N], F32, name="s_row", bufs=1)
            nc.vector.tensor_mul(s_row[:], pi_row[:], qr_row[:])
            s_ps = psp.tile([128, JB], F32, name="s_ps", tag="tp", bufs=2)
            for j in range(JB):
                nc.tensor.transpose(s_ps[:, j:j + 1], s_row[:, j * 128:(j + 1) * 128],
                                    eye[0:1, 0:1])
            s_col = wrk.tile([128, JB], F32R, name="s_col", bufs=1)
            nc.scalar.copy(s_col[:], s_ps[:])

            # ---- ysf[j][p, c] = s_j * x[c, j]  (PE transpose + scaled evac)
            ysf = [wrk.tile([128, C], F32R, name="ysf", bufs=9) for _ in range(JB)]
            for cb in range(CB):
                for j in range(JB):
                    t_ps = psp.tile([128, 128], F32, name="t_ps", tag="tp", bufs=2)
                    nc.tensor.matmul(t_ps[:],
                                     xb_t[cb][:, j * 128:(j + 1) * 128],
                                     eye[:],
                                     start=True, stop=True, is_transpose=True)
                    dst = ysf[j][:, cb * 128:(cb + 1) * 128]
                    sc = s_col[:, j:j + 1].bitcast(F32)
                    if (cb * JB + j) % 2 == 0:
                        nc.scalar.mul(dst, t_ps[:], sc)
                    else:
                        nc.vector.tensor_scalar_mul(dst, t_ps[:], sc)

            # ---- dE[n] = sum_j s_j Ku[j, n] ; d = r*dE + 1e-5 ; v = 0.12*r/d
            d_ps = [psp.tile([1, 512], F32, name="d_ps", tag="vec") for _ in range(NBL)]
            for nb in range(NBL):
                for j in range(JB):
                    nc.tensor.matmul(d_ps[nb][:], s_col[:, j:j + 1],
                                     ku[j][:, _sl(nb)],
                                     start=(j == 0), stop=(j == JB - 1))
            dr_row = wrk.tile([1, N], F32, name="rtmp", bufs=2)
            for nb in range(NBL):
                nc.vector.tensor_tensor(dr_row[:, _sl(nb)], r_row[:, _sl(nb)],
                                        d_ps[nb][:], op=ALU.mult)
            d_row = wrk.tile([1, N], F32, name="rtmp", bufs=2)
            nc.vector.tensor_scalar_add(d_row[:], dr_row[:], 1e-5)
            dinv_row = wrk.tile([1, N], F32, name="rtmp", bufs=2)
            nc.vector.reciprocal(dinv_row[:], d_row[:])
            v_row = wrk.tile([1, N], F32, name="rtmp", bufs=2)
            nc.vector.scalar_tensor_tensor(v_row[:], r_row[:], 0.12, dinv_row[:],
                                           op0=ALU.mult, op1=ALU.mult)
            vbc = wrk.tile([128, N], F32, name="vbc", bufs=1)
            nc.gpsimd.partition_broadcast(vbc[:], v_row[:])

            # ---- out[c, n] = 0.97*x[c, n] + v_n * sum_j ysf[j, c] * Ku[j, n]
            for cb in range(CB):
                out_t = wrk.tile([128, N], F32, name="out_t", bufs=2)
                for nb in range(NBL):
                    m_ps = psp.tile([128, 512], F32, name="m_ps", tag="m")
                    for j in range(JB):
                        nc.tensor.matmul(m_ps[:],
                                         ysf[j][:, cb * 128:(cb + 1) * 128],
                                         ku[j][:, _sl(nb)],
                                         start=(j == 0), stop=(j == JB - 1))
                    tmul = wrk.tile([128, 512], F32, name="tmul", bufs=2)
                    nc.vector.tensor_mul(tmul[:], m_ps[:], vbc[:, _sl(nb)])
                    nc.vector.scalar_tensor_tensor(out_t[:, _sl(nb)],
                                                   xb_t[cb][:, _sl(nb)], 0.97, tmul[:],
                                                   op0=ALU.mult, op1=ALU.add)
                nc.sync.dma_start(out_d[s, cb * 128:(cb + 1) * 128, :], out_t[:])

    nc.compile()
    return nc


_NC_CACHE = {}


def _get_nc():
    if "nc" not in _NC_CACHE:
        _NC_CACHE["nc"] = build_nc()
    return _NC_CACHE["nc"]


def make_in_maps(x, x_original, proj_w, proj_b, pi_w1, pi_b1, pi_w2, pi_b2):
    xs = np.ascontiguousarray(np.asarray(x, dtype=np.float32)[:, 0])   # (B, C, N)
    xos = np.ascontiguousarray(np.asarray(x_original, dtype=np.float32))
    proj_w = np.asarray(proj_w, dtype=np.float32)
    pwt = np.ascontiguousarray(proj_w.T.reshape(CB, 128, L).transpose(1, 0, 2).reshape(128, CB * L))
    pbc = np.ascontiguousarray(np.asarray(proj_b, dtype=np.float32).reshape(L, 1))
    w1t = np.ascontiguousarray(np.asarray(pi_w1, dtype=np.float32).T)  # (16, 768)
    b1c = np.ascontiguousarray(np.asarray(pi_b1, dtype=np.float32).reshape(CB, 128).T)
    w2c = np.ascontiguousarray(np.asarray(pi_w2, dtype=np.float32).reshape(CB, 128).T)
    b2c = np.asarray(pi_b2, dtype=np.float32).reshape(1, 1)
    eye = np.eye(128, dtype=np.float32)
    ones = np.ones((128, 1), dtype=np.float32)
    in_maps = []
    for core in range(NCORES):
        sl = slice(SPC * core, SPC * (core + 1))
        in_maps.append({
            "xo": np.ascontiguousarray(xos[sl]),
            "xb": np.ascontiguousarray(xs[sl]),
            "pwt": pwt, "pbc": pbc, "w1t": w1t, "b1c": b1c,
            "w2c": w2c, "b2c": b2c, "eye": eye, "ones": ones,
        })
    return in_maps


def run(inputs, trace=False):
    nc = _get_nc()
    in_maps = make_in_maps(**inputs)
    res = run_bass_kernel_spmd(nc, in_maps, list(range(NCORES)), trace=trace)
    out = np.concatenate([res.results[i]["out"] for i in range(NCORES)], axis=0)
    return out.astype(np.float32), res


def kernel(**inputs):
    out, _ = run(inputs, trace=False)
    return out


# revision 16
# speedup vs baseline: 501.1181x; 501.1181x over previous
import sys

if "/opt/trn_rl_repo" not in sys.path:
    sys.path.insert(0, "/opt/trn_rl_repo")

import numpy as np
from contextlib import ExitStack

import concourse.bass as bass
from concourse import bacc
import concourse.tile as tile
from concourse import mybir
from concourse.bass_utils import run_bass_kernel_spmd

B, C, N, L = 16, 768, 1024, 16
SPC = 2          # samples per core
NCORES = 8
CB = 6           # 128-row chunks of C
JB = 8           # 128-row chunks of N (contraction j)
NBL = 2          # 512-col blocks of N
F32 = mybir.dt.float32
F32R = mybir.dt.float32r
AF = mybir.ActivationFunctionType
ALU = mybir.AluOpType

USE_F32R = True


def _mm(ap):
    return ap.bitcast(F32R) if USE_F32R else ap


def _sl(nb):
    return slice(nb * 512, (nb + 1) * 512)


def build_nc(reps=1):
    nc = bacc.Bacc(trn_type="TRN2")
    xo_d = nc.declare_dram_parameter("xo", [SPC, C, N], F32, isOutput=False)
    xb_d = nc.declare_dram_parameter("xb", [SPC, C, N], F32, isOutput=False)
    pwt_d = nc.declare_dram_parameter("pwt", [128, CB * L], F32, isOutput=False)
    pbc_d = nc.declare_dram_parameter("pbc", [L, 1], F32, isOutput=False)
    w1t_d = nc.declare_dram_parameter("w1t", [L, C], F32, isOutput=False)
    b1c_d = nc.declare_dram_parameter("b1c", [128, CB], F32, isOutput=False)
    w2c_d = nc.declare_dram_parameter("w2c", [128, CB], F32, isOutput=False)
    b2c_d = nc.declare_dram_parameter("b2c", [1, 1], F32, isOutput=False)
    eye_d = nc.declare_dram_parameter("eye", [128, 128], F32, isOutput=False)
    ones_d = nc.declare_dram_parameter("ones", [128, 1], F32, isOutput=False)
    out_d = nc.declare_dram_parameter("out", [SPC, C, N], F32, isOutput=True)

    with tile.TileContext(nc) as tc, ExitStack() as ctx:
        con = ctx.enter_context(tc.tile_pool(name="con", bufs=1))
        wrk = ctx.enter_context(tc.tile_pool(name="wrk", bufs=2))
        psp = ctx.enter_context(tc.tile_pool(name="psp", bufs=2, space="PSUM"))

        pwt = con.tile([128, CB * L], F32)
        pbc = con.tile([L, 1], F32)
        w1t = con.tile([L, C], F32)
        b1c = con.tile([128, CB], F32)
        w2c = con.tile([128, CB], F32)
        b2c = con.tile([1, 1], F32)
        eye = con.tile([128, 128], F32)
        ones = con.tile([128, 1], F32)
        for t, d in ((pwt, pwt_d), (pbc, pbc_d), (w1t, w1t_d), (b1c, b1c_d),
                     (w2c, w2c_d), (b2c, b2c_d), (eye, eye_d), (ones, ones_d)):
            nc.sync.dma_start(t[:], d[:])
        # f32r-rounded copies (BIR verifier: f32r matmul operands must be
        # produced by a compute op with f32r output dtype)
        pwt_r = con.tile([128, CB * L], F32R)
        w1t_r = con.tile([L, C], F32R)
        w2c_r = con.tile([128, CB], F32R)
        ones_r = con.tile([128, 1], F32R)
        for t, src in ((pwt_r, pwt), (w1t_r, w1t), (w2c_r, w2c), (ones_r, ones)):
            nc.gpsimd.tensor_copy(t[:], src[:])

        for s in [s for _ in range(reps) for s in range(SPC)]:
            # ---- zT = proj_w @ xo + b : (16, N); xo streamed + rounded to f32r
            z_ps = [psp.tile([L, 512], F32, name="z_ps", tag="vec") for _ in range(NBL)]
            for cb in range(CB):
                xot = wrk.tile([128, N], F32, name="xo_t", bufs=3)
                nc.sync.dma_start(xot[:], xo_d[s, cb * 128:(cb + 1) * 128, :])
                xor_ = wrk.tile([128, N], F32R, name="xo_r", bufs=3)
                nc.gpsimd.tensor_copy(xor_[:], xot[:])
                for nb in range(NBL):
                    nc.tensor.matmul(
                        z_ps[nb][:],
                        pwt_r[:, cb * L:(cb + 1) * L],
                        xor_[:, _sl(nb)],
                        start=(cb == 0), stop=(cb == CB - 1))
            xb_t = []
            for cb in range(CB):
                xbt = wrk.tile([128, N], F32, name="xb_t", bufs=6)
                nc.sync.dma_start(xbt[:], xb_d[s, cb * 128:(cb + 1) * 128, :])
                xb_t.append(xbt)
            zT = wrk.tile([L, N], F32R, name="zT", bufs=2)
            for nb in range(NBL):
                nc.scalar.activation(zT[:, _sl(nb)], z_ps[nb][:], AF.Identity,
                                     bias=pbc[:], scale=1.0)

            # ---- sq(n) = sum_l zT^2 ; nsq_row = -sq ; r = exp(-sq)
            zsq = wrk.tile([L, N], F32R, name="zsq", bufs=1)
            nc.vector.tensor_mul(zsq[:], zT[:].bitcast(F32), zT[:].bitcast(F32))
            sq_ps = [psp.tile([1, 512], F32, name="sq_ps", tag="vec") for _ in range(NBL)]
            for nb in range(NBL):
                nc.tensor.matmul(sq_ps[nb][:], ones_r[0:L, :], zsq[:, _sl(nb)],
                                 start=True, stop=True)
            nsq_row = wrk.tile([1, N], F32, name="nsq_row", bufs=1)
            for nb in range(NBL):
                nc.vector.tensor_scalar_mul(nsq_row[:, _sl(nb)], sq_ps[nb][:], -1.0)
            nsq_ps = psp.tile([128, JB], F32, name="nsq_ps", tag="tp", bufs=2)
            for j in range(JB):
                nc.tensor.transpose(nsq_ps[:, j:j + 1], nsq_row[:, j * 128:(j + 1) * 128],
                                    eye[0:1, 0:1])
            nsq_col = wrk.tile([128, JB], F32, name="nsq_col", bufs=1)
            nc.scalar.copy(nsq_col[:], nsq_ps[:])
            r_row = wrk.tile([1, N], F32, name="r_row", bufs=1)
            nc.scalar.activation(r_row[:], nsq_row[:], AF.Exp)

            # ---- Gram + Ku[j] = exp(2*G - sq_j)  (row-stabilized kernel)
            ku = [wrk.tile([128, N], F32R, name="ku", bufs=9) for _ in range(JB)]
            for j in range(JB):
                for nb in range(NBL):
                    g_ps = psp.tile([128, 512], F32, name="g_ps", tag="g")
                    nc.tensor.matmul(g_ps[:], zT[:, j * 128:(j + 1) * 128],
                                     zT[:, _sl(nb)], start=True, stop=True)
                    nc.scalar.activation(ku[j][:, _sl(nb)], g_ps[:], AF.Exp,
                                         bias=nsq_col[:, j:j + 1], scale=2.0)

            # ---- qE[n] = sum_j Ku[j, n] (column sums)
            q_ps = [psp.tile([1, 512], F32, name="q_ps", tag="vec") for _ in range(NBL)]
            for nb in range(NBL):
                for j in range(JB):
                    nc.tensor.matmul(q_ps[nb][:], ones_r[:], ku[j][:, _sl(nb)],
                                     start=(j == 0), stop=(j == JB - 1))

            # ---- pi MLP: relu(z @ w1.T + b1) @ w2.T + b2 -> sigmoid
            pi_ps = [psp.tile([1, 512], F32, name="pi_ps", tag="vec") for _ in range(NBL)]
            for cb in range(CB):
                h_ps = [psp.tile([128, 512], F32, name="h_ps", tag="g") for _ in range(NBL)]
                for nb in range(NBL):
                    nc.tensor.matmul(h_ps[nb][:], w1t_r[:, cb * 128:(cb + 1) * 128],
                                     zT[:, _sl(nb)], start=True, stop=True)
                h_sb = wrk.tile([128, N], F32R, name="h_sb", bufs=2)
                for nb in range(NBL):
                    nc.vector.tensor_scalar(h_sb[:, _sl(nb)], h_ps[nb][:],
                                            b1c[:, cb:cb + 1], 0.0,
                                            op0=ALU.add, op1=ALU.max)
                for nb in range(NBL):
                    nc.tensor.matmul(pi_ps[nb][:], w2c_r[:, cb:cb + 1],
                                     h_sb[:, _sl(nb)],
                                     start=(cb == 0), stop=(cb == CB - 1))
            pi_row = wrk.tile([1, N], F32, name="pi_row", bufs=1)
            for nb in range(NBL):
                nc.scalar.activation(pi_row[:, _sl(nb)], pi_ps[nb][:], AF.Sigmoid,
                                     bias=b2c[:], scale=1.0)

            # ---- s = pi / (r * qE)
            q_row = wrk.tile([1, N], F32, name="rtmp", bufs=2)
            for nb in range(NBL):
                nc.vector.tensor_tensor(q_row[:, _sl(nb)], r_row[:, _sl(nb)],
                                        q_ps[nb][:], op=ALU.mult)
            qr_row = wrk.tile([1, N], F32, name="rtmp", bufs=2)
            nc.vector.reciprocal(qr_row[:], q_row[:])
            s_row = wrk.tile([1, N], F32, name="s_row", bufs=1)
            nc.vector.tensor_mul(s_row[:], pi_row[:], qr_row[:])
            s_ps = psp.tile([128, JB], F32, name="s_ps", tag="tp", bufs=2)
            for j in range(JB):
                nc.tensor.transpose(s_ps[:, j:j + 1], s_row[:, j * 128:(j + 1) * 128],
                                    eye[0:1, 0:1])
            s_col = wrk.tile([128, JB], F32R, name="s_col", bufs=1)
            nc.scalar.copy(s_col[:], s_ps[:])

            # ---- ysf[j][p, c] = s_j * x[c, j]  (PE transpose + scaled evac)
            ysf = [wrk.tile([128, C], F32R, name="ysf", bufs=9) for _ in range(JB)]
            for cb in range(CB):
                for j in range(JB):
                    t_ps = psp.tile([128, 128], F32, name="t_ps", tag="tp", bufs=2)
                    nc.tensor.matmul(t_ps[:],
                                     xb_t[cb][:, j * 128:(j + 1) * 128],
                                     eye[:],
                                     start=True, stop=True, is_transpose=True)
                    dst = ysf[j][:, cb * 128:(cb + 1) * 128]
                    sc = s_col[:, j:j + 1].bitcast(F32)
                    if (cb * JB + j) % 2 == 0:
                        nc.scalar.mul(dst, t_ps[:], sc)
                    else:
                        nc.vector.tensor_scalar_mul(dst, t_ps[:], sc)

            # ---- dE[n] = sum_j s_j Ku[j, n] ; d = r*dE + 1e-5 ; v = 0.12*r/d
            d_ps = [psp.tile([1, 512], F32, name="d_ps", tag="vec") for _ in range(NBL)]
            for nb in range(NBL):
                for j in range(JB):
                    nc.tensor.matmul(d_ps[nb][:], s_col[:, j:j + 1],
                                     ku[j][:, _sl(nb)],
                                     start=(j == 0), stop=(j == JB - 1))
            dr_row = wrk.tile([1, N], F32, name="rtmp", bufs=2)
            for nb in range(NBL):
                nc.vector.tensor_tensor(dr_row[:, _sl(nb)], r_row[:, _sl(nb)],
                                        d_ps[nb][:], op=ALU.mult)
            d_row = wrk.tile([1, N], F32, name="rtmp", bufs=2)
            nc.vector.tensor_scalar_add(d_row[:], dr_row[:], 1e-5)
            dinv_row = wrk.tile([1, N], F32, name="rtmp", bufs=2)
            nc.vector.reciprocal(dinv_row[:], d_row[:])
            v_row = wrk.tile([1, N], F32, name="rtmp", bufs=2)
            nc.vector.scalar_tensor_tensor(v_row[:], r_row[:], 0.12, dinv_row[:],
                                           op0=ALU.mult, op1=ALU.mult)
            vbc = wrk.tile([128, N], F32, name="vbc", bufs=1)
            nc.gpsimd.partition_broadcast(vbc[:], v_row[:])

            # ---- out[c, n] = 0.97*x[c, n] + v_n * sum_j ysf[j, c] * Ku[j, n]
            for cb in range(CB):
                out_t = wrk.tile([128, N], F32, name="out_t", bufs=2)
                for nb in range(NBL):
                    m_ps = psp.tile([128, 512], F32, name="m_ps", tag="m")
                    for j in range(JB):
                        nc.tensor.matmul(m_ps[:],
                                         ysf[j][:, cb * 128:(cb + 1) * 128],
                                         ku[j][:, _sl(nb)],
                                         start=(j == 0), stop=(j == JB - 1))
                    tmul = wrk.tile([128, 512], F32, name="tmul", bufs=2)
                    nc.vector.tensor_mul(tmul[:], m_ps[:], vbc[:, _sl(nb)])
                    nc.vector.scalar_tensor_tensor(out_t[:, _sl(nb)],
                                                   xb_t[cb][:, _sl(nb)], 0.97, tmul[:],
                                                   op0=ALU.mult, op1=ALU.add)
                nc.sync.dma_start(out_d[s, cb * 128:(cb + 1) * 128, :], out_t[:])

    nc.compile()
    return nc


_NC_CACHE = {}


def _get_nc(reps=1):
    if reps not in _NC_CACHE:
        _NC_CACHE[reps] = build_nc(reps)
    return _NC_CACHE[reps]


def make_in_maps(x, x_original, proj_w, proj_b, pi_w1, pi_b1, pi_w2, pi_b2):
    xs = np.ascontiguousarray(np.asarray(x, dtype=np.float32)[:, 0])   # (B, C, N)
    xos = np.ascontiguousarray(np.asarray(x_original, dtype=np.float32))
    proj_w = np.asarray(proj_w, dtype=np.float32)
    pwt = np.ascontiguousarray(proj_w.T.reshape(CB, 128, L).transpose(1, 0, 2).reshape(128, CB * L))
    pbc = np.ascontiguousarray(np.asarray(proj_b, dtype=np.float32).reshape(L, 1))
    w1t = np.ascontiguousarray(np.asarray(pi_w1, dtype=np.float32).T)  # (16, 768)
    b1c = np.ascontiguousarray(np.asarray(pi_b1, dtype=np.float32).reshape(CB, 128).T)
    w2c = np.ascontiguousarray(np.asarray(pi_w2, dtype=np.float32).reshape(CB, 128).T)
    b2c = np.asarray(pi_b2, dtype=np.float32).reshape(1, 1)
    eye = np.eye(128, dtype=np.float32)
    ones = np.ones((128, 1), dtype=np.float32)
    in_maps = []
    for core in range(NCORES):
        sl = slice(SPC * core, SPC * (core + 1))
        in_maps.append({
            "xo": np.ascontiguousarray(xos[sl]),
            "xb": np.ascontiguousarray(xs[sl]),
            "pwt": pwt, "pbc": pbc, "w1t": w1t, "b1c": b1c,
            "w2c": w2c, "b2c": b2c, "eye": eye, "ones": ones,
        })
    return in_maps


def run(inputs, trace=False):
    nc = _get_nc()
    in_maps = make_in_maps(**inputs)
    res = run_bass_kernel_spmd(nc, in_maps, list(range(NCORES)), trace=trace)
    out = np.concatenate([res.results[i]["out"] for i in range(NCORES)], axis=0)
    return out.astype(np.float32), res


def kernel(**inputs):
    out, _ = run(inputs, trace=False)
    return out
